# revision 12
# baseline (speedup 1.0000x reference)
"""Trainium2 Bass kernel for DeformableSincConv1d (v2, fp16 data path).

Data parallel over batch: 4 rows/core on 8 cores. Per core:
  1. Windowed im2col load (fp16): XX[l, j] = x_pad[10*l + j], j in [0,53)
  2. PE transposes (fp16 ident) -> psum [53, 1024] groups; evacuation copies
     apply the +1/+2 partition shifts: psum[1:52] -> X0P, psum[2:53] -> XPP.
     Batch-pair rows packed at partition bases 0 and 64 (legal SBUF starts).
  3. Offset conv: one matmul per 512-chunk, lhsT block-diag [115,115] fp16;
     Act evacuates psum with bias -> QS fp16 (chunked).
  4. Sampling, chunked pair-packed [115, 512] fp16: clip (Pool), mask=is_ge
     (DVE), Ep = XPP-X0P chunk (DVE), Em = DMA partition-shift of Ep,
     copy_predicated, mul, per-row add -> dd[0:51]; then dd[51:102] = DMA
     column-shift (stacked-102 rhs for the rotated-filter decomposition).
  5. Final conv: t0-outer per row, 51 rotated-filter matmuls (fp16, 102
     contraction); single strided psum->ysb copies spread over DVE/Act/Pool.
"""

import sys

import numpy as np

if "/opt/trn_rl_repo" not in sys.path:
    sys.path.insert(0, "/opt/trn_rl_repo")

SR = 16000
C_OUT = 80
K = 51
STRIDE = 10
HALF = (K - 1) // 2

B_FULL = 32
N_CORES = 8
B_LOC = B_FULL // N_CORES
L_FULL = 32000

R1 = 64          # partition base of second row in pair tiles
NP = R1 + K      # 115 rows in pair tiles


def _derive(L):
    L_out = (L - K) // STRIDE + 1
    T_out = (L_out * K - K) // STRIDE + 1
    NCHUNK = (L_out + 127) // 128
    LPAD = NCHUNK * 128
    XLEN = 10 * LPAD + 48
    return L_out, T_out, NCHUNK, LPAD, XLEN


def _host_filters(hz, band):
    hzc = np.clip(hz.astype(np.float32), 0.0, SR / 2).astype(np.float32)
    bandc = np.clip(band.astype(np.float32), 3.0, SR / 2).astype(np.float32)
    t_right = (np.arange(1, HALF + 1, dtype=np.float32) / np.float32(SR)).astype(np.float32)
    low = (hzc - bandc / 2).astype(np.float32)
    high = (hzc + bandc / 2).astype(np.float32)

    def sinc(t):
        ts = np.where(t == 0, np.float32(1.0), t)
        return np.where(t == 0, np.float32(1.0), np.sin(ts) / ts).astype(np.float32)

    a1 = (2 * high).astype(np.float32)
    a2 = (2 * low).astype(np.float32)
    bp_left = (a1 * sinc(a1 * t_right) - a2 * sinc(a2 * t_right)).astype(np.float32)
    bp = np.concatenate([bp_left, np.ones((C_OUT, 1), np.float32), bp_left[:, ::-1]], axis=1)
    return (bp / (2 * bandc)).astype(np.float32)  # [C_OUT, K]


def _host_f102(filt, L):
    """Stacked rotated filter matrices [128, K*C_OUT]; rows 0..50 = A-half
    (column offset a), rows 51..101 = B-half (column offset a+1, served by the
    column-shifted copy at dd[51:102])."""
    L_out, T_out, _, LPAD, _ = _derive(L)
    F = np.zeros((128, K, C_OUT), np.float32)
    for t0 in range(K):
        a = (STRIDE * t0) // K
        ns = (T_out - 1 - t0) // K + 1
        for k2 in range(K):
            kstar = (k2 + STRIDE * t0) % K
            lstar = (STRIDE * t0 + k2) // K
            if lstar == a:
                F[kstar, t0, :] = filt[:, k2]
            else:
                assert lstar == a + 1
                F[51 + kstar, t0, :] = filt[:, k2]
        assert a + 1 + STRIDE * (ns - 1) <= L_out - 1
        assert a + STRIDE * (ns - 1) <= LPAD - 1
    return F.reshape(128, K * C_OUT)


def build_program(B_loc=B_LOC, L=L_FULL, debug=False):
    import concourse.bacc as bacc
    import concourse.tile as tile
    from concourse import bass, mybir

    f32 = mybir.dt.float32
    f16 = mybir.dt.float16
    u8 = mybir.dt.uint8
    Alu = mybir.AluOpType
    Act = mybir.ActivationFunctionType

    L_out, T_out, NCHUNK, LPAD, XLEN = _derive(L)
    CC = 512
    NCC = (L_out + CC - 1) // CC
    NG = (NCHUNK + 7) // 8          # transpose psum groups of 8 chunks
    n_pairs = B_loc // 2
    NSMAX = (T_out - 1) // K + 1

    nc = bacc.Bacc("TRN2", target_bir_lowering=False, debug=debug)

    x_d = nc.dram_tensor("x", [B_loc, XLEN], f16, kind="ExternalInput")
    wr2_d = nc.dram_tensor("wr2", [NP, NP], f16, kind="ExternalInput")
    offb2_d = nc.dram_tensor("offb2", [NP, 1], f32, kind="ExternalInput")
    lovec2_d = nc.dram_tensor("lovec2", [NP, 1], f32, kind="ExternalInput")
    hivec2_d = nc.dram_tensor("hivec2", [NP, 1], f32, kind="ExternalInput")
    f102_d = nc.dram_tensor("f102", [128, K * C_OUT], f16, kind="ExternalInput")
    ident_d = nc.dram_tensor("ident", [128, 128], f16, kind="ExternalInput")
    y_d = nc.dram_tensor("y", [B_loc, C_OUT, T_out], f32, kind="ExternalOutput")

    xap = x_d[:]

    with tile.TileContext(nc) as tc:
        with (
            tc.tile_pool(name="consts", bufs=1) as consts,
            tc.tile_pool(name="xxp", bufs=1) as xxp,
            tc.tile_pool(name="xkp", bufs=2) as xkp,
            tc.tile_pool(name="x0p", bufs=1) as x0p,
            tc.tile_pool(name="xpp", bufs=1) as xpp,
            tc.tile_pool(name="qsp", bufs=2) as qsp,
            tc.tile_pool(name="mtp", bufs=2) as mtp,
            tc.tile_pool(name="epp", bufs=2) as epp,
            tc.tile_pool(name="emp", bufs=2) as emp,
            tc.tile_pool(name="ddp", bufs=3) as ddp,
            tc.tile_pool(name="ysbp", bufs=2) as ysbp,
            tc.tile_pool(name="tpsum", bufs=2, space="PSUM") as tpsum,
            tc.tile_pool(name="qpsum", bufs=2, space="PSUM") as qpsum,
            tc.tile_pool(name="fpsum", bufs=4, space="PSUM") as fpsum,
        ):
            wr2_sb = consts.tile([NP, NP], f16)
            nc.sync.dma_start(out=wr2_sb[:], in_=wr2_d[:])
            offb2_sb = consts.tile([NP, 1], f32)
            nc.sync.dma_start(out=offb2_sb[:], in_=offb2_d[:])
            lovec2_sb = consts.tile([NP, 1], f32)
            nc.sync.dma_start(out=lovec2_sb[:], in_=lovec2_d[:])
            hivec2_sb = consts.tile([NP, 1], f32)
            nc.sync.dma_start(out=hivec2_sb[:], in_=hivec2_d[:])
            f102_sb = consts.tile([128, K * C_OUT], f16)
            nc.sync.dma_start(out=f102_sb[:], in_=f102_d[:])
            ident_sb = consts.tile([128, 128], f16)
            nc.sync.dma_start(out=ident_sb[:], in_=ident_d[:])

            evac_engines = [nc.vector, nc.scalar]

            def ecopy(eng, dst, src):
                if eng is nc.scalar:
                    eng.copy(dst, src)
                else:
                    eng.tensor_copy(dst, src)

            def front_pair(p):
                r0 = 2 * p
                X0P = x0p.tile([NP, LPAD], f16)
                XPP = xpp.tile([NP, LPAD], f16)
                ei = 0
                for ri in range(2):
                    base = 0 if ri == 0 else R1
                    xx = xxp.tile([128, NCHUNK, 53], f16, tag="xx")
                    nh1 = NCHUNK // 2
                    for h0, hn in ((0, nh1), (nh1, NCHUNK - nh1)):
                        in_ap = bass.AP(
                            tensor=xap.tensor,
                            offset=(r0 + ri) * XLEN + 1280 * h0,
                            ap=[[10, 128], [1280, hn], [1, 53]],
                        )
                        nc.sync.dma_start(out=xx[:, h0:h0 + hn, :], in_=in_ap)
                    for g in range(NG):
                        n = min(8, NCHUNK - 8 * g)
                        pt = tpsum.tile([53, 1024], f16)
                        for c in range(n):
                            nc.tensor.transpose(pt[:, c * 128:(c + 1) * 128],
                                                xx[:, 8 * g + c, :], ident_sb[:])
                        cols = slice(g * 1024, g * 1024 + n * 128)
                        xk = xkp.tile([53, 1024], f16)
                        ecopy(evac_engines[ei % 2], xk[:, :n * 128], pt[:, :n * 128])
                        ei += 1
                        nc.sync.dma_start(out=X0P[base:base + K, cols],
                                          in_=xk[1:52, :n * 128])
                        nc.sync.dma_start(out=XPP[base:base + K, cols],
                                          in_=xk[2:53, :n * 128])

                # fill never-written pair-gap rows (51..63) with finite junk so
                # the 115-row matmul/elementwise ops never see NaN bit patterns
                gap = R1 - K
                nc.sync.dma_start(out=X0P[K:R1, :], in_=X0P[0:gap, :])
                nc.sync.dma_start(out=XPP[K:R1, :], in_=XPP[0:gap, :])

                dd0 = ddp.tile([102, LPAD], f16, tag="dd")
                dd1 = ddp.tile([102, LPAD], f16, tag="dd")

                for c7 in range(NCC):
                    n = min(CC, L_out - c7 * CC)
                    sl = slice(c7 * CC, c7 * CC + n)
                    qp = qpsum.tile([NP, CC], f32)
                    nc.tensor.matmul(qp[:, :n], wr2_sb[:], X0P[:, sl],
                                     start=True, stop=True)
                    QSC = qsp.tile([NP, CC], f16)
                    nc.scalar.activation(QSC[:, :n], qp[:, :n], Act.Identity,
                                         bias=offb2_sb[:])
                    nc.gpsimd.tensor_scalar(QSC[:, :n], QSC[:, :n],
                                            lovec2_sb[:], hivec2_sb[:],
                                            op0=Alu.max, op1=Alu.min)
                    MTC = mtp.tile([NP, CC], u8)
                    nc.gpsimd.tensor_scalar(MTC[:, :n], QSC[:, :n], 0.0, None,
                                            op0=Alu.is_ge)
                    EPC = epp.tile([NP, CC], f16)
                    nc.gpsimd.tensor_sub(EPC[:, :n], XPP[:, sl], X0P[:, sl])
                    EMC = emp.tile([NP, CC], f16)
                    nc.sync.dma_start(out=EMC[0:1, :n], in_=EPC[0:1, :n])
                    nc.sync.dma_start(out=EMC[1:K, :n], in_=EPC[0:K - 1, :n])
                    nc.sync.dma_start(out=EMC[K:R1 + 1, :n], in_=EPC[K:R1 + 1, :n])
                    nc.sync.dma_start(out=EMC[R1 + 1:NP, :n], in_=EPC[R1:NP - 1, :n])
                    nc.vector.copy_predicated(EMC[:, :n], MTC[:, :n], EPC[:, :n])
                    nc.gpsimd.tensor_mul(EMC[:, :n], QSC[:, :n], EMC[:, :n])
                    nc.vector.tensor_add(dd0[0:K, sl], X0P[0:K, sl], EMC[0:K, :n])
                    nc.vector.tensor_add(dd1[0:K, sl], X0P[R1:NP, sl], EMC[R1:NP, :n])

                nc.sync.dma_start(out=dd0[K:102, 0:L_out - 1], in_=dd0[0:K, 1:L_out])
                nc.sync.dma_start(out=dd1[K:102, 0:L_out - 1], in_=dd1[0:K, 1:L_out])
                return dd0, dd1

            SCATTER = {
                0: [nc.scalar, nc.vector, nc.scalar],
                1: [nc.scalar, nc.vector, nc.scalar],
                2: [nc.scalar, nc.vector, nc.scalar],
                3: [nc.scalar, nc.vector, nc.scalar],
            }

            def final_row(r, dd):
                ysb = ysbp.tile([C_OUT, T_out], f32)
                pat = SCATTER[r % 4]
                for t0 in range(K):
                    a = (STRIDE * t0) // K
                    ns = (T_out - 1 - t0) // K + 1
                    fp = fpsum.tile([C_OUT, NSMAX], f32)
                    rhs = dd[0:102, a:a + STRIDE * (ns - 1) + 1:STRIDE]
                    lhsT = f102_sb[0:102, t0 * C_OUT:(t0 + 1) * C_OUT]
                    nc.tensor.matmul(fp[:, :ns], lhsT, rhs, start=True, stop=True)
                    yv = ysb[:, t0:t0 + K * (ns - 1) + 1:K]
                    ecopy(pat[t0 % len(pat)], yv, fp[:, :ns])
                nc.sync.dma_start(out=y_d[r], in_=ysb[:])

            dd0, dd1 = front_pair(0)
            final_row(0, dd0)
            dd2, dd3 = front_pair(1)
            final_row(1, dd1)
            final_row(2, dd2)
            final_row(3, dd3)

    nc.compile()
    return nc


def _host_inputs(x, hz, band, offset_w, offset_b, B_loc, L):
    """Build the per-core input maps."""
    L_out, T_out, NCHUNK, LPAD, XLEN = _derive(L)
    filt = _host_filters(hz, band)
    f102 = _host_f102(filt, L).astype(np.float16)
    wr = offset_w[:, 0, :].T.astype(np.float32)  # [k_in, k_out]
    wr2 = np.zeros((NP, NP), np.float32)
    wr2[0:K, 0:K] = wr
    wr2[R1:NP, R1:NP] = wr
    offb2 = np.zeros((NP, 1), np.float32)
    offb2[0:K, 0] = offset_b.astype(np.float32)
    offb2[R1:NP, 0] = offset_b.astype(np.float32)
    kk = np.arange(K, dtype=np.float32)
    lovec2 = np.zeros((NP, 1), np.float32)
    hivec2 = np.zeros((NP, 1), np.float32)
    lovec2[0:K, 0] = -kk
    lovec2[R1:NP, 0] = -kk
    hivec2[0:K, 0] = 50.0 - kk
    hivec2[R1:NP, 0] = 50.0 - kk
    ident = np.eye(128, dtype=np.float16)

    B = x.shape[0]
    xpad = np.zeros((B, XLEN), np.float16)
    xpad[:, 1:1 + L] = x.astype(np.float16)

    n_cores = B // B_loc
    in_maps = []
    for i in range(n_cores):
        in_maps.append({
            "x": np.ascontiguousarray(xpad[i * B_loc:(i + 1) * B_loc]),
            "wr2": wr2.astype(np.float16),
            "offb2": offb2,
            "lovec2": lovec2,
            "hivec2": hivec2,
            "f102": f102,
            "ident": ident,
        })
    return in_maps


_CACHED = {}


def _get_program():
    key = (B_LOC, L_FULL)
    if key not in _CACHED:
        _CACHED[key] = build_program(B_LOC, L_FULL)
    return _CACHED[key]


def kernel(x, hz, band, offset_w, offset_b):
    from concourse.bass_utils import run_bass_kernel_spmd

    x = np.asarray(x, dtype=np.float32)
    hz = np.asarray(hz, dtype=np.float32)
    band = np.asarray(band, dtype=np.float32)
    offset_w = np.asarray(offset_w, dtype=np.float32)
    offset_b = np.asarray(offset_b, dtype=np.float32)

    nc = _get_program()
    in_maps = _host_inputs(x, hz, band, offset_w, offset_b, B_LOC, L_FULL)
    res = run_bass_kernel_spmd(nc, in_maps, list(range(N_CORES)))
    outs = [res.results[i]["y"] for i in range(N_CORES)]
    return np.concatenate(outs, axis=0)


# revision 14
# speedup vs baseline: 1.3843x; 1.3843x over previous
"""Trainium2 Bass kernel for DeformableSincConv1d (v2, fp16 data path).

Data parallel over batch: 4 rows/core on 8 cores. Per core:
  1. Windowed im2col load (fp16): XX[l, j] = x_pad[10*l + j], j in [0,53)
  2. PE transposes (fp16 ident) -> psum [53, 1024] groups; evacuation copies
     apply the +1/+2 partition shifts: psum[1:52] -> X0P, psum[2:53] -> XPP.
     Batch-pair rows packed at partition bases 0 and 64 (legal SBUF starts).
  3. Offset conv: one matmul per 510-chunk, lhsT block-diag [115,115] fp16;
     Act evacuates psum with bias -> QS fp16; mask = is_ge(psum, -offb) on
     DVE straight from psum (no separate clip: max|offset| < 1, and the
     k=0 / k=50 boundary clips are realized exactly by zeroing Ep[50] and
     Em[0] rows).
  4. Sampling, chunked pair-packed [115, 510] fp16 with Pool doing the
     tensor_tensor work: Ep = XPP-X0P, Em = DMA partition-shift of Ep,
     copy_predicated (DVE), mul, then per-row adds whose destination AP
     performs the polyphase split: dd[k, r, s] = D[k, 10s + r].
  5. Final conv: t0-outer per row, 51 rotated-filter matmuls with fp16
     CONTIGUOUS rhs dd[0:102, a, :ns]; dd[51:102] = plane-rotated DMA copy
     (stacked-102 rotated-filter decomposition); single strided psum->ysb
     copies alternating DVE/Act.
"""

import sys

import numpy as np

if "/opt/trn_rl_repo" not in sys.path:
    sys.path.insert(0, "/opt/trn_rl_repo")

SR = 16000
C_OUT = 80
K = 51
STRIDE = 10
HALF = (K - 1) // 2

B_FULL = 32
N_CORES = 8
B_LOC = B_FULL // N_CORES
L_FULL = 32000

R1 = 64          # partition base of second row in pair tiles
NP = R1 + K      # 115 rows in pair tiles


def _derive(L):
    L_out = (L - K) // STRIDE + 1
    T_out = (L_out * K - K) // STRIDE + 1
    NCHUNK = (L_out + 127) // 128
    LPAD = NCHUNK * 128
    XLEN = 10 * LPAD + 48
    return L_out, T_out, NCHUNK, LPAD, XLEN


def _host_filters(hz, band):
    hzc = np.clip(hz.astype(np.float32), 0.0, SR / 2).astype(np.float32)
    bandc = np.clip(band.astype(np.float32), 3.0, SR / 2).astype(np.float32)
    t_right = (np.arange(1, HALF + 1, dtype=np.float32) / np.float32(SR)).astype(np.float32)
    low = (hzc - bandc / 2).astype(np.float32)
    high = (hzc + bandc / 2).astype(np.float32)

    def sinc(t):
        ts = np.where(t == 0, np.float32(1.0), t)
        return np.where(t == 0, np.float32(1.0), np.sin(ts) / ts).astype(np.float32)

    a1 = (2 * high).astype(np.float32)
    a2 = (2 * low).astype(np.float32)
    bp_left = (a1 * sinc(a1 * t_right) - a2 * sinc(a2 * t_right)).astype(np.float32)
    bp = np.concatenate([bp_left, np.ones((C_OUT, 1), np.float32), bp_left[:, ::-1]], axis=1)
    return (bp / (2 * bandc)).astype(np.float32)  # [C_OUT, K]


def _host_f102(filt, L):
    """Stacked rotated filter matrices [128, K*C_OUT]; rows 0..50 = A-half
    (column offset a), rows 51..101 = B-half (column offset a+1, served by the
    column-shifted copy at dd[51:102])."""
    L_out, T_out, _, LPAD, _ = _derive(L)
    F = np.zeros((128, K, C_OUT), np.float32)
    for t0 in range(K):
        a = (STRIDE * t0) // K
        ns = (T_out - 1 - t0) // K + 1
        for k2 in range(K):
            kstar = (k2 + STRIDE * t0) % K
            lstar = (STRIDE * t0 + k2) // K
            if lstar == a:
                F[kstar, t0, :] = filt[:, k2]
            else:
                assert lstar == a + 1
                F[51 + kstar, t0, :] = filt[:, k2]
        assert a + 1 + STRIDE * (ns - 1) <= L_out - 1
        assert a + STRIDE * (ns - 1) <= STRIDE * ((T_out - 1) // K + 1) - 1
    return F.reshape(128, K * C_OUT)


def build_program(B_loc=B_LOC, L=L_FULL, debug=False):
    import concourse.bacc as bacc
    import concourse.tile as tile
    from concourse import bass, mybir

    f32 = mybir.dt.float32
    f16 = mybir.dt.float16
    u8 = mybir.dt.uint8
    Alu = mybir.AluOpType
    Act = mybir.ActivationFunctionType

    L_out, T_out, NCHUNK, LPAD, XLEN = _derive(L)
    NSMAX = (T_out - 1) // K + 1
    NPL = NSMAX                   # polyphase plane length (s slots)
    LSAMP = STRIDE * NPL          # sampled deformed region (covers all reads)
    assert LSAMP <= LPAD
    CC = 510
    NCC = (LSAMP + CC - 1) // CC
    NG = (NCHUNK + 7) // 8          # transpose psum groups of 8 chunks
    n_pairs = B_loc // 2

    nc = bacc.Bacc("TRN2", target_bir_lowering=False, debug=debug)

    x_d = nc.dram_tensor("x", [B_loc, XLEN], f16, kind="ExternalInput")
    wr2_d = nc.dram_tensor("wr2", [NP, NP], f16, kind="ExternalInput")
    offb2_d = nc.dram_tensor("offb2", [NP, 1], f32, kind="ExternalInput")
    negoffb2_d = nc.dram_tensor("negoffb2", [NP, 1], f32, kind="ExternalInput")
    f102_d = nc.dram_tensor("f102", [128, K * C_OUT], f16, kind="ExternalInput")
    ident_d = nc.dram_tensor("ident", [128, 128], f16, kind="ExternalInput")
    y_d = nc.dram_tensor("y", [B_loc, C_OUT, T_out], f32, kind="ExternalOutput")

    xap = x_d[:]

    with tile.TileContext(nc) as tc:
        with (
            tc.tile_pool(name="consts", bufs=1) as consts,
            tc.tile_pool(name="xxp", bufs=1) as xxp,
            tc.tile_pool(name="xkp", bufs=2) as xkp,
            tc.tile_pool(name="x0p", bufs=1) as x0p,
            tc.tile_pool(name="xpp", bufs=1) as xpp,
            tc.tile_pool(name="qsp", bufs=2) as qsp,
            tc.tile_pool(name="mtp", bufs=2) as mtp,
            tc.tile_pool(name="epp", bufs=2) as epp,
            tc.tile_pool(name="emp", bufs=2) as emp,
            tc.tile_pool(name="ddp", bufs=3) as ddp,
            tc.tile_pool(name="ysbp", bufs=2) as ysbp,
            tc.tile_pool(name="tpsum", bufs=2, space="PSUM") as tpsum,
            tc.tile_pool(name="qpsum", bufs=2, space="PSUM") as qpsum,
            tc.tile_pool(name="fpsum", bufs=4, space="PSUM") as fpsum,
        ):
            wr2_sb = consts.tile([NP, NP], f16)
            nc.sync.dma_start(out=wr2_sb[:], in_=wr2_d[:])
            offb2_sb = consts.tile([NP, 1], f32)
            nc.sync.dma_start(out=offb2_sb[:], in_=offb2_d[:])
            negoffb2_sb = consts.tile([NP, 1], f32)
            nc.sync.dma_start(out=negoffb2_sb[:], in_=negoffb2_d[:])
            f102_sb = consts.tile([128, K * C_OUT], f16)
            nc.sync.dma_start(out=f102_sb[:], in_=f102_d[:])
            ident_sb = consts.tile([128, 128], f16)
            nc.sync.dma_start(out=ident_sb[:], in_=ident_d[:])

            evac_engines = [nc.vector, nc.scalar]

            def ecopy(eng, dst, src):
                if eng is nc.scalar:
                    eng.copy(dst, src)
                else:
                    eng.tensor_copy(dst, src)

            def front_pair(p):
                r0 = 2 * p
                X0P = x0p.tile([NP, LPAD], f16)
                XPP = xpp.tile([NP, LPAD], f16)
                ei = 0
                for ri in range(2):
                    base = 0 if ri == 0 else R1
                    xx = xxp.tile([128, NCHUNK, 53], f16, tag="xx")
                    nh1 = NCHUNK // 2
                    for h0, hn in ((0, nh1), (nh1, NCHUNK - nh1)):
                        in_ap = bass.AP(
                            tensor=xap.tensor,
                            offset=(r0 + ri) * XLEN + 1280 * h0,
                            ap=[[10, 128], [1280, hn], [1, 53]],
                        )
                        nc.sync.dma_start(out=xx[:, h0:h0 + hn, :], in_=in_ap)
                    for g in range(NG):
                        n = min(8, NCHUNK - 8 * g)
                        pt = tpsum.tile([53, 1024], f16)
                        for c in range(n):
                            nc.tensor.transpose(pt[:, c * 128:(c + 1) * 128],
                                                xx[:, 8 * g + c, :], ident_sb[:])
                        cols = slice(g * 1024, g * 1024 + n * 128)
                        xk = xkp.tile([53, 1024], f16)
                        ecopy(evac_engines[ei % 2], xk[:, :n * 128], pt[:, :n * 128])
                        ei += 1
                        nc.sync.dma_start(out=X0P[base:base + K, cols],
                                          in_=xk[1:52, :n * 128])
                        nc.sync.dma_start(out=XPP[base:base + K, cols],
                                          in_=xk[2:53, :n * 128])

                # fill never-written pair-gap rows (51..63) with finite junk so
                # the 115-row matmul/elementwise ops never see NaN bit patterns
                gap = R1 - K
                nc.sync.dma_start(out=X0P[K:R1, :], in_=X0P[0:gap, :])
                nc.sync.dma_start(out=XPP[K:R1, :], in_=XPP[0:gap, :])

                # boundary clip realized as zeros: Ep[k=50] rows become 0
                nc.sync.dma_start(out=XPP[K - 1:K, :], in_=X0P[K - 1:K, :])
                nc.sync.dma_start(out=XPP[NP - 1:NP, :], in_=X0P[NP - 1:NP, :])

                dd0 = ddp.tile([102, STRIDE, NPL], f16, tag="dd")
                dd1 = ddp.tile([102, STRIDE, NPL], f16, tag="dd")

                for c7 in range(NCC):
                    n = min(CC, LSAMP - c7 * CC)
                    sl = slice(c7 * CC, c7 * CC + n)
                    s0, nS = c7 * CC // STRIDE, n // STRIDE
                    qp = qpsum.tile([NP, CC], f32)
                    nc.tensor.matmul(qp[:, :n], wr2_sb[:], X0P[:, sl],
                                     start=True, stop=True)
                    QSC = qsp.tile([NP, CC], f16)
                    nc.scalar.activation(QSC[:, :n], qp[:, :n], Act.Identity,
                                         bias=offb2_sb[:])
                    MTC = mtp.tile([NP, CC], u8)
                    nc.vector.tensor_scalar(MTC[:, :n], qp[:, :n],
                                            negoffb2_sb[:], None, op0=Alu.is_ge)
                    EPC = epp.tile([NP, CC], f16)
                    nc.gpsimd.tensor_sub(EPC[:, :n], XPP[:, sl], X0P[:, sl])
                    EMC = emp.tile([NP, CC], f16)
                    nc.sync.dma_start(out=EMC[0:1, :n], in_=EPC[K - 1:K, :n])
                    nc.sync.dma_start(out=EMC[1:K, :n], in_=EPC[0:K - 1, :n])
                    nc.sync.dma_start(out=EMC[K:R1, :n], in_=EPC[K:R1, :n])
                    nc.sync.dma_start(out=EMC[R1:R1 + 1, :n], in_=EPC[NP - 1:NP, :n])
                    nc.sync.dma_start(out=EMC[R1 + 1:NP, :n], in_=EPC[R1:NP - 1, :n])
                    nc.vector.copy_predicated(EMC[:, :n], MTC[:, :n], EPC[:, :n])
                    nc.gpsimd.tensor_mul(EMC[:, :n], QSC[:, :n], EMC[:, :n])
                    dv0 = dd0[0:K, :, s0:s0 + nS].rearrange("p r s -> p s r")
                    dv1 = dd1[0:K, :, s0:s0 + nS].rearrange("p r s -> p s r")
                    nc.gpsimd.tensor_add(dv0, X0P[0:K, sl], EMC[0:K, :n])
                    nc.gpsimd.tensor_add(dv1, X0P[R1:NP, sl], EMC[R1:NP, :n])

                # B-half: dd[51+k, r, s] = D[k, 10s + r + 1] via plane rotation
                nc.sync.dma_start(out=dd0[K:102, 0:9, :], in_=dd0[0:K, 1:10, :])
                nc.sync.dma_start(out=dd0[K:102, 9, 0:NPL - 1], in_=dd0[0:K, 0, 1:NPL])
                nc.sync.dma_start(out=dd1[K:102, 0:9, :], in_=dd1[0:K, 1:10, :])
                nc.sync.dma_start(out=dd1[K:102, 9, 0:NPL - 1], in_=dd1[0:K, 0, 1:NPL])
                return dd0, dd1

            SCATTER = {
                0: [nc.scalar, nc.vector, nc.scalar],
                1: [nc.scalar, nc.vector, nc.scalar],
                2: [nc.scalar, nc.vector, nc.scalar],
                3: [nc.scalar, nc.vector, nc.scalar],
            }

            def final_row(r, dd):
                ysb = ysbp.tile([C_OUT, T_out], f32)
                pat = SCATTER[r % 4]
                for t0 in range(K):
                    a = (STRIDE * t0) // K
                    ns = (T_out - 1 - t0) // K + 1
                    fp = fpsum.tile([C_OUT, NSMAX], f32)
                    rhs = dd[0:102, a, 0:ns]
                    lhsT = f102_sb[0:102, t0 * C_OUT:(t0 + 1) * C_OUT]
                    nc.tensor.matmul(fp[:, :ns], lhsT, rhs, start=True, stop=True)
                    yv = ysb[:, t0:t0 + K * (ns - 1) + 1:K]
                    ecopy(pat[t0 % len(pat)], yv, fp[:, :ns])
                nc.sync.dma_start(out=y_d[r], in_=ysb[:])

            dd0, dd1 = front_pair(0)
            final_row(0, dd0)
            dd2, dd3 = front_pair(1)
            final_row(1, dd1)
            final_row(2, dd2)
            final_row(3, dd3)

    nc.compile()
    return nc


def _host_inputs(x, hz, band, offset_w, offset_b, B_loc, L):
    """Build the per-core input maps."""
    L_out, T_out, NCHUNK, LPAD, XLEN = _derive(L)
    filt = _host_filters(hz, band)
    f102 = _host_f102(filt, L).astype(np.float16)
    wr = offset_w[:, 0, :].T.astype(np.float32)  # [k_in, k_out]
    wr2 = np.zeros((NP, NP), np.float32)
    wr2[0:K, 0:K] = wr
    wr2[R1:NP, R1:NP] = wr
    offb2 = np.zeros((NP, 1), np.float32)
    offb2[0:K, 0] = offset_b.astype(np.float32)
    offb2[R1:NP, 0] = offset_b.astype(np.float32)
    negoffb2 = -offb2
    ident = np.eye(128, dtype=np.float16)

    B = x.shape[0]
    xpad = np.zeros((B, XLEN), np.float16)
    xpad[:, 1:1 + L] = x.astype(np.float16)

    n_cores = B // B_loc
    in_maps = []
    for i in range(n_cores):
        in_maps.append({
            "x": np.ascontiguousarray(xpad[i * B_loc:(i + 1) * B_loc]),
            "wr2": wr2.astype(np.float16),
            "offb2": offb2,
            "negoffb2": negoffb2,
            "f102": f102,
            "ident": ident,
        })
    return in_maps


_CACHED = {}


def _get_program():
    key = (B_LOC, L_FULL)
    if key not in _CACHED:
        _CACHED[key] = build_program(B_LOC, L_FULL)
    return _CACHED[key]


def kernel(x, hz, band, offset_w, offset_b):
    from concourse.bass_utils import run_bass_kernel_spmd

    x = np.asarray(x, dtype=np.float32)
    hz = np.asarray(hz, dtype=np.float32)
    band = np.asarray(band, dtype=np.float32)
    offset_w = np.asarray(offset_w, dtype=np.float32)
    offset_b = np.asarray(offset_b, dtype=np.float32)

    nc = _get_program()
    in_maps = _host_inputs(x, hz, band, offset_w, offset_b, B_LOC, L_FULL)
    res = run_bass_kernel_spmd(nc, in_maps, list(range(N_CORES)))
    outs = [res.results[i]["y"] for i in range(N_CORES)]
    return np.concatenate(outs, axis=0)
